# revision 1
# baseline (speedup 1.0000x reference)
"""Trainium2 Bass kernel for nn_KuramotoCoupling.

Reference computation (B=2, S=2048, D=1024, H=16, HD=64):
    mag   = sqrt(z_re^2 + z_im^2)
    q,k,v = mag @ W{q,k,v}.T  (per-head, HD=64)
    w     = softmax(q k^T / 8)
    value_mix    = (w @ v) @ Wo.T
    phase_update = c * (cos_ph * (w @ sin_ph) - sin_ph * (w @ cos_ph))
where sin_ph = z_im / mag, cos_ph = z_re / mag  (since ph = atan2(z_im, z_re)).

Sharding: 8 cores = 2 batches x 4 head-groups (4 heads each).  Each core
gets its batch's z (full D, needed for the QKV contraction), its heads'
slices of Wq/Wk/Wv (pre-transposed on host), and its j-slice of Wo^T.
Per-core outputs: phase slice [S, 256] (concatenated on host) and a
value_mix partial [S, D] (summed over the 4 head-groups of a batch on
host -- the standard row-parallel TP unshard).

Device-side structure per core:
  A) stream z by 128-row chunks: ssq = re^2+im^2; PE-transpose ssq and
     evict through ACT Sqrt -> magT [d, s] (fp32r); sin/cos of the own
     256 channels -> X tiles [s, (head: v|sin|cos|ones64)] (fp32r).
  B) qT/kT = WqT.T @ magT  [ch, s] (bf16), v = magT.T @ WvT -> X v cols.
  C) per head, per 1024-wide q-tile: scoresT[k,q] = kT.T @ qT (bf16 mms),
     exp via ACT (scale=1/8, no max-sub: |scores/8| < ~6) -> wT (fp32r);
     out[q, v|sin|cos|ones] accumulates wT.T @ X over k (fp32r, N=256).
     The ones column gives the softmax denominator for free.
     recip = 1/sum; value rows scaled by recip; phase = (Ksin*rc).cos -
     (Kcos*rc).sin with rc = recip*coupling (fused scalar_tensor_tensor).
  D) PE-transpose the normalized value rows -> voutT [j, s]; value_mix
     partial [s, i] = voutT.T @ WoT (fp32r); DMA out.
"""
import sys

if "/opt/trn_rl_repo" not in sys.path:
    sys.path.insert(0, "/opt/trn_rl_repo")

import numpy as np
from contextlib import ExitStack

import concourse.bacc as bacc
import concourse.mybir as mybir
import concourse.tile as tile
from concourse.bass_utils import run_bass_kernel_spmd
from concourse.masks import make_identity

F32 = mybir.dt.float32
F32R = mybir.dt.float32r
BF16 = mybir.dt.bfloat16
AF = mybir.ActivationFunctionType
ALU = mybir.AluOpType

B, S, D, H = 2, 2048, 1024, 16
HD = D // H
N_CORES = 8
HG = 4                   # head-groups (tensor-parallel dim)
HPC = H // HG            # heads per core = 4
CH = HPC * HD            # own channels per core = 256
P = 128
SC = S // P              # 16 s-chunks
DC = D // P              # 8 d-chunks
KC = S // P              # 16 k-chunks
XW = HPC * 256           # X tile width: per head [v64|sin64|cos64|ones64]


def build_kernel(sc=SC, attn_bf16=False):
    """Build the SPMD program for one core.  sc = number of 128-row
    s-chunks (16 for the real problem; smaller for debug builds).
    attn_bf16: run the attention-mixing path (wT, X) in bf16 -- faster,
    slightly less accurate; scores qT/kT are bf16 either way."""
    s = sc * P
    XDT = BF16 if attn_bf16 else F32R
    XN = 193 if attn_bf16 else 256   # 2nd-matmul moving width (fp32r needs >=256)
    kc = sc                      # k-chunks == s-chunks
    halves = ((0, sc // 2), (sc // 2, sc)) if sc >= 4 else ((0, sc),)
    qtiles = []                  # (q0_chunk, q1_chunk) 8-chunk tiles
    step = min(8, sc)
    for t0 in range(0, sc, step):
        qtiles.append((t0, min(t0 + step, sc)))

    nc = bacc.Bacc("TRN2", target_bir_lowering=False, debug=False,
                   num_devices=N_CORES)

    z_re = nc.dram_tensor("z_re", [s, D], F32, kind="ExternalInput").ap()
    z_im = nc.dram_tensor("z_im", [s, D], F32, kind="ExternalInput").ap()
    z_re_o = nc.dram_tensor("z_re_o", [s, CH], F32, kind="ExternalInput").ap()
    z_im_o = nc.dram_tensor("z_im_o", [s, CH], F32, kind="ExternalInput").ap()
    wq_t = nc.dram_tensor("wq_t", [D, CH], F32, kind="ExternalInput").ap()
    wk_t = nc.dram_tensor("wk_t", [D, CH], F32, kind="ExternalInput").ap()
    wv_t = nc.dram_tensor("wv_t", [D, CH], F32, kind="ExternalInput").ap()
    wo_t = nc.dram_tensor("wo_t", [CH, D], F32, kind="ExternalInput").ap()
    coup = nc.dram_tensor("coup", [P, 1], F32, kind="ExternalInput").ap()
    phase_out = nc.dram_tensor("phase_out", [s, CH], F32,
                               kind="ExternalOutput").ap()
    vmix_out = nc.dram_tensor("vmix_out", [s, D], F32,
                              kind="ExternalOutput").ap()

    with ExitStack() as ctx:
        tc = ctx.enter_context(tile.TileContext(nc))

        # ---- persistent pools -------------------------------------------
        const_p = ctx.enter_context(tc.tile_pool(name="const", bufs=1))
        wo_pool = ctx.enter_context(tc.tile_pool(name="wo", bufs=1))
        x_pool = ctx.enter_context(tc.tile_pool(name="x", bufs=1))
        qk_pool = ctx.enter_context(tc.tile_pool(name="qk", bufs=1))

        ident = const_p.tile([P, P], F32, tag="ident")
        make_identity(nc, ident[:])
        coup_sb = const_p.tile([P, 1], F32, tag="coup")
        nc.sync.dma_start(coup_sb[:], coup[:, :])

        wo_sb = wo_pool.tile([P, 2 * D], F32R, tag="wo")
        nc.sync.dma_start(
            wo_sb[:].rearrange("p (j c) -> p j c", c=D),
            wo_t.rearrange("(j p) c -> p j c", p=P).bitcast(F32R))

        # X tiles: per s-chunk [128, XW] fp32r, per head [v|sin|cos|ones64]
        x_tiles = [x_pool.tile([P, XW], XDT, tag=f"x{i}", name=f"x{i}")
                   for i in range(sc)]
        # qT/kT: [ch, s] bf16, 2 chunk-tiles each (heads 0,1 / heads 2,3)
        qT = [qk_pool.tile([P, s], BF16, tag=f"qT{c}", name=f"qT{c}")
              for c in range(2)]
        kT = [qk_pool.tile([P, s], BF16, tag=f"kT{c}", name=f"kT{c}")
              for c in range(2)]
        if attn_bf16:
            sc_pool = ctx.enter_context(tc.tile_pool(name="sincos", bufs=1))
            sinF = [sc_pool.tile([P, CH], F32, tag=f"sinF{i}", name=f"sinF{i}")
                    for i in range(sc)]
            cosF = [sc_pool.tile([P, CH], F32, tag=f"cosF{i}", name=f"cosF{i}")
                    for i in range(sc)]

        # ---- phase A/B: mag, transposes, QKV ----------------------------
        with tc.tile_pool(name="wqkv", bufs=1) as w_pool, \
             tc.tile_pool(name="zb", bufs=2) as z_pool, \
             tc.tile_pool(name="zo", bufs=2) as zo_pool, \
             tc.tile_pool(name="sq", bufs=2) as sq_pool, \
             tc.tile_pool(name="ssq", bufs=3) as ssq_pool, \
             tc.tile_pool(name="rv", bufs=2) as rv_pool, \
             tc.tile_pool(name="magt", bufs=DC + 1) as mag_pool, \
             tc.tile_pool(name="tp", bufs=3, space="PSUM") as tp_pool, \
             tc.tile_pool(name="pqk", bufs=3, space="PSUM") as pqk_pool, \
             tc.tile_pool(name="pv", bufs=2, space="PSUM") as pv_pool:

            # weights: one DMA each, chunked [128, .] by contraction rows
            wq_sb = w_pool.tile([P, DC * CH], F32R, tag="wq")
            wk_sb = w_pool.tile([P, DC * CH], F32R, tag="wk")
            wv_sb = w_pool.tile([P, DC * CH], F32R, tag="wv")
            for wsb, wdr in ((wq_sb, wq_t), (wk_sb, wk_t), (wv_sb, wv_t)):
                nc.sync.dma_start(
                    wsb[:].rearrange("p (j c) -> p j c", c=CH),
                    wdr.rearrange("(j p) c -> p j c", p=P).bitcast(F32R))

            for h0, h1 in halves:
                nh = h1 - h0
                ssq_tiles = {}
                # -- elementwise + sin/cos per s-chunk
                for si in range(h0, h1):
                    zr = z_pool.tile([P, D], F32, tag="zr")
                    zi = z_pool.tile([P, D], F32, tag="zi")
                    nc.sync.dma_start(zr[:], z_re[si * P:(si + 1) * P, :])
                    nc.sync.dma_start(zi[:], z_im[si * P:(si + 1) * P, :])
                    t1 = sq_pool.tile([P, D], F32, tag="t1")
                    t2 = sq_pool.tile([P, D], F32, tag="t2")
                    nc.gpsimd.tensor_mul(t1[:], zr[:], zr[:])
                    nc.vector.tensor_mul(t2[:], zi[:], zi[:])
                    ssq = ssq_pool.tile([P, D], F32, tag="ssq")
                    nc.vector.tensor_add(ssq[:], t1[:], t2[:])
                    ssq_tiles[si] = ssq

                    # own-channel slices -> sin/cos/ones into X
                    zro = zo_pool.tile([P, CH], F32, tag="zro")
                    zio = zo_pool.tile([P, CH], F32, tag="zio")
                    nc.sync.dma_start(zro[:], z_re_o[si * P:(si + 1) * P, :])
                    nc.sync.dma_start(zio[:], z_im_o[si * P:(si + 1) * P, :])
                    t1o = rv_pool.tile([P, CH], F32, tag="t1o")
                    t2o = rv_pool.tile([P, CH], F32, tag="t2o")
                    sso = rv_pool.tile([P, CH], F32, tag="sso")
                    msl = rv_pool.tile([P, CH], F32, tag="msl")
                    rinv = rv_pool.tile([P, CH], F32, tag="rinv")
                    nc.gpsimd.tensor_mul(t1o[:], zro[:], zro[:])
                    nc.vector.tensor_mul(t2o[:], zio[:], zio[:])
                    nc.vector.tensor_add(sso[:], t1o[:], t2o[:])
                    nc.scalar.activation(msl[:], sso[:], AF.Sqrt)
                    nc.vector.reciprocal_approx_fast(out=rinv[:], in_=msl[:])

                    x3 = x_tiles[si][:].rearrange("p (h c) -> p h c", c=256)
                    z3r = zro[:].rearrange("p (h c) -> p h c", c=HD)
                    z3i = zio[:].rearrange("p (h c) -> p h c", c=HD)
                    r3 = rinv[:].rearrange("p (h c) -> p h c", c=HD)
                    if attn_bf16:
                        nc.vector.tensor_mul(sinF[si][:], zio[:], rinv[:])
                        nc.vector.tensor_mul(cosF[si][:], zro[:], rinv[:])
                        s3 = sinF[si][:].rearrange("p (h c) -> p h c", c=HD)
                        c3 = cosF[si][:].rearrange("p (h c) -> p h c", c=HD)
                        nc.vector.tensor_copy(x3[:, :, 64:128], s3[:])
                        nc.vector.tensor_copy(x3[:, :, 128:192], c3[:])
                    else:
                        nc.vector.tensor_mul(x3[:, :, 64:128], z3i[:], r3[:])
                        nc.vector.tensor_mul(x3[:, :, 128:192], z3r[:], r3[:])
                    nc.vector.tensor_scalar(
                        out=x3[:, :, 192:256], in0=r3[:],
                        scalar1=0.0, scalar2=1.0, op0=ALU.mult, op1=ALU.add)

                # -- transposes of ssq -> magT (ACT sqrt evict), groups of 2
                mag_t = [mag_pool.tile([P, nh * P], F32R, tag="magt",
                                        name=f"magt{j}")
                         for j in range(DC)]
                for g in range(0, nh, 2):
                    sa, sb_ = h0 + g, h0 + g + 1
                    for j in range(DC):
                        tp = tp_pool.tile([P, 256], F32, tag="tp")
                        nc.tensor.transpose(
                            tp[:, 0:128], ssq_tiles[sa][:, j * P:(j + 1) * P],
                            ident[:])
                        if sb_ < h1:
                            nc.tensor.transpose(
                                tp[:, 128:256],
                                ssq_tiles[sb_][:, j * P:(j + 1) * P], ident[:])
                        nc.scalar.activation(
                            mag_t[j][:, g * P:(g + 2) * P], tp[:], AF.Sqrt)

                # -- qT / kT for this half
                for c in range(2):
                    for t0 in range(0, nh * P, 512):
                        tw = min(512, nh * P - t0)
                        for wsb, dst in ((wq_sb, qT[c]), (wk_sb, kT[c])):
                            ps = pqk_pool.tile([P, 512], F32, tag="pqk")
                            for j in range(DC):
                                nc.tensor.matmul(
                                    ps[:, 0:tw],
                                    wsb[:, j * CH + c * P:j * CH + (c + 1) * P],
                                    mag_t[j][:, t0:t0 + tw],
                                    start=(j == 0), stop=(j == DC - 1))
                            nc.vector.tensor_copy(
                                dst[:, h0 * P + t0:h0 * P + t0 + tw],
                                ps[:, 0:tw])

                # -- v for this half -> X v columns
                for ii in range(nh):
                    si = h0 + ii
                    psv = pv_pool.tile([P, CH], F32, tag="pv")
                    for j in range(DC):
                        nc.tensor.matmul(
                            psv[:], mag_t[j][:, ii * P:(ii + 1) * P],
                            wv_sb[:, j * CH:(j + 1) * CH],
                            start=(j == 0), stop=(j == DC - 1))
                    x3 = x_tiles[si][:].rearrange("p (h c) -> p h c", c=256)
                    p3 = psv[:].rearrange("p (h c) -> p h c", c=HD)
                    nc.vector.tensor_copy(x3[:, :, 0:HD], p3[:])

        # ---- phase C/D: attention + output ------------------------------
        with tc.tile_pool(name="wt", bufs=min(kc, 16)) as wt_pool, \
             tc.tile_pool(name="vout", bufs=min(sc, 8) + 2) as vout_pool, \
             tc.tile_pool(name="vT", bufs=2) as vT_pool, \
             tc.tile_pool(name="ph", bufs=min(sc, 8) + 2) as ph_pool, \
             tc.tile_pool(name="vmix", bufs=2) as vmix_pool, \
             tc.tile_pool(name="rc", bufs=8) as rc_pool, \
             tc.tile_pool(name="tmp", bufs=4) as tmp_pool, \
             tc.tile_pool(name="pss", bufs=2, space="PSUM") as pss_pool, \
             tc.tile_pool(name="pso", bufs=2, space="PSUM") as pso_pool, \
             tc.tile_pool(name="pst", bufs=1, space="PSUM") as pst_pool, \
             tc.tile_pool(name="psm", bufs=1, space="PSUM") as psm_pool:

            for (t0, t1) in qtiles:
                qw = (t1 - t0) * P      # q-tile width (<= 1024)
                vout_tiles = [vout_pool.tile([P, CH], F32R, tag="vout",
                                             name=f"vout{u}")
                              for u in range(t1 - t0)]
                ph_tiles = [ph_pool.tile([P, CH], F32, tag="ph", name=f"ph{u}")
                            for u in range(t1 - t0)]

                head_groups = ((0, 1), (2, 3)) if attn_bf16 else \
                    ((0,), (1,), (2,), (3,))
                for hgrp in head_groups:
                  # pass A: scoresT -> exp -> wT (per k-chunk); heads of a
                  # pair run concurrently in the PE via row-groups 0-63/64-127
                  wts_h = {h: [] for h in hgrp}
                  for k in range(kc):
                    for h in hgrp:
                        c, hh = divmod(h, 2)
                        kTh, qTh = kT[c], qT[c]
                        ps_s = pss_pool.tile([P, qw], F32, tag="pss",
                                             name=f"pss{h % 2}")
                        for q0 in range(0, qw, 512):
                            qn = min(512, qw - q0)
                            nc.tensor.matmul(
                                ps_s[:, q0:q0 + qn],
                                kTh[hh * 64:(hh + 1) * 64, k * P:(k + 1) * P],
                                qTh[hh * 64:(hh + 1) * 64,
                                    t0 * P + q0:t0 * P + q0 + qn],
                                start=True, stop=True)
                        wtag = f"wt{h % 2}" if attn_bf16 else "wt"
                        wt = wt_pool.tile([P, qw], XDT, tag=wtag,
                                          name=f"wt{h % 2}")
                        nc.scalar.activation(wt[:], ps_s[:], AF.Exp,
                                             scale=0.125)
                        wts_h[h].append(wt)
                  for h in hgrp:
                    wts = wts_h[h]
                    # pass B: out[q, v|sin|cos|ones] = wT.T @ X_h over k
                    for u in range(t1 - t0):
                        ps_o = pso_pool.tile([P, 256], F32, tag="pso")
                        for k in range(kc):
                            nc.tensor.matmul(
                                ps_o[:, 0:XN],
                                wts[k][:, u * P:(u + 1) * P],
                                x_tiles[k][:, h * 256:h * 256 + XN],
                                start=(k == 0), stop=(k == kc - 1))
                        # softmax denominator -> recip, rc = recip * coup
                        rec = rc_pool.tile([P, 1], F32, tag="rec")
                        rc = rc_pool.tile([P, 1], F32, tag="rc")
                        nc.vector.reciprocal(rec[:], ps_o[:, 192:193])
                        nc.vector.tensor_scalar_mul(rc[:], rec[:], coup_sb[:])
                        # normalized value rows
                        nc.vector.tensor_scalar(
                            out=vout_tiles[u][:, h * HD:(h + 1) * HD],
                            in0=ps_o[:, 0:HD], scalar1=rec[:], scalar2=None,
                            op0=ALU.mult)
                        # phase: (Ksin*rc).cos - (Kcos*rc).sin
                        si = t0 + u
                        if attn_bf16:
                            cos_src = cosF[si][:, h * HD:(h + 1) * HD]
                            sin_src = sinF[si][:, h * HD:(h + 1) * HD]
                        else:
                            x3 = x_tiles[si][:].rearrange(
                                "p (h c) -> p h c", c=256)
                            cos_src = x3[:, h, 128:192].bitcast(F32)
                            sin_src = x3[:, h, 64:128].bitcast(F32)
                        ta = tmp_pool.tile([P, HD], F32, tag="ta")
                        tb = tmp_pool.tile([P, HD], F32, tag="tb")
                        nc.vector.scalar_tensor_tensor(
                            out=ta[:], in0=ps_o[:, 64:128], scalar=rc[:],
                            in1=cos_src, op0=ALU.mult, op1=ALU.mult)
                        nc.vector.scalar_tensor_tensor(
                            out=tb[:], in0=ps_o[:, 128:192], scalar=rc[:],
                            in1=sin_src, op0=ALU.mult, op1=ALU.mult)
                        nc.vector.tensor_sub(
                            ph_tiles[u][:, h * HD:(h + 1) * HD], ta[:], tb[:])

                # phase D for this q-tile: transpose vout, Wo matmul, DMA
                for u in range(t1 - t0):
                    nc.sync.dma_start(
                        phase_out[(t0 + u) * P:(t0 + u + 1) * P, :],
                        ph_tiles[u][:])

                vT = [vT_pool.tile([P, qw], F32R, tag=f"vT{c}", name=f"vT{c}")
                      for c in range(2)]
                for c in range(2):
                    for g in range(0, t1 - t0, 4):
                        gn = min(4, t1 - t0 - g)
                        tp = pst_pool.tile([P, 512], F32, tag="pst")
                        for u in range(g, g + gn):
                            nc.tensor.transpose(
                                tp[:, (u - g) * P:(u - g + 1) * P],
                                vout_tiles[u][:, c * P:(c + 1) * P]
                                .bitcast(F32),
                                ident[:])
                        nc.vector.tensor_copy(
                            vT[c][:, g * P:(g + gn) * P],
                            tp[:, 0:gn * P])
                for u in range(t1 - t0):
                    su = t0 + u
                    vm = vmix_pool.tile([P, D], F32, tag="vm")
                    for it in range(2):
                        psm = psm_pool.tile([P, 512], F32, tag="psm")
                        for c in range(2):
                            nc.tensor.matmul(
                                psm[:],
                                vT[c][:, u * P:(u + 1) * P],
                                wo_sb[:, c * D + it * 512:c * D + it * 512 + 512],
                                start=(c == 0), stop=(c == 1))
                        nc.vector.tensor_copy(
                            vm[:, it * 512:(it + 1) * 512], psm[:])
                    nc.sync.dma_start(vmix_out[su * P:(su + 1) * P, :], vm[:])

    if not nc.is_finalized():
        nc.finalize()
    return nc


_CACHED = {}
ATTN_BF16 = False


def _get_nc(sc=SC, attn_bf16=None):
    if attn_bf16 is None:
        attn_bf16 = ATTN_BF16
    key = (sc, attn_bf16)
    if key not in _CACHED:
        _CACHED[key] = build_kernel(sc, attn_bf16)
    return _CACHED[key]


def make_in_maps(z_re, z_im, Wq, Wk, Wv, Wo, coupling_strength, sc=SC):
    """Host-side sharding: core = b * HG + hg."""
    z_re = np.asarray(z_re, dtype=np.float32)
    z_im = np.asarray(z_im, dtype=np.float32)
    coup_b = np.full((P, 1), np.float32(np.asarray(coupling_strength)),
                     dtype=np.float32)
    s = sc * P
    in_maps = []
    for core in range(N_CORES):
        b, hg = divmod(core, HG)
        lo, hi = hg * CH, (hg + 1) * CH
        in_maps.append({
            "z_re": np.ascontiguousarray(z_re[b, :s]),
            "z_im": np.ascontiguousarray(z_im[b, :s]),
            "z_re_o": np.ascontiguousarray(z_re[b, :s, lo:hi]),
            "z_im_o": np.ascontiguousarray(z_im[b, :s, lo:hi]),
            "wq_t": np.ascontiguousarray(np.asarray(Wq)[lo:hi, :].T),
            "wk_t": np.ascontiguousarray(np.asarray(Wk)[lo:hi, :].T),
            "wv_t": np.ascontiguousarray(np.asarray(Wv)[lo:hi, :].T),
            "wo_t": np.ascontiguousarray(np.asarray(Wo)[:, lo:hi].T),
            "coup": coup_b,
        })
    return in_maps


def gather_outputs(results, sc=SC):
    s = sc * P
    phase = np.zeros((B, s, D), dtype=np.float32)
    vmix = np.zeros((B, s, D), dtype=np.float32)
    for core in range(N_CORES):
        b, hg = divmod(core, HG)
        lo, hi = hg * CH, (hg + 1) * CH
        phase[b, :, lo:hi] = results[core]["phase_out"]
        vmix[b] += results[core]["vmix_out"]
    return phase, vmix


def kernel(z_re, z_im, Wq, Wk, Wv, Wo, coupling_strength):
    nc = _get_nc(SC)
    in_maps = make_in_maps(z_re, z_im, Wq, Wk, Wv, Wo, coupling_strength)
    res = run_bass_kernel_spmd(nc, in_maps, core_ids=list(range(N_CORES)))
    return gather_outputs(res.results)



# revision 5
# speedup vs baseline: 12.5963x; 12.5963x over previous
"""Trainium2 Bass kernel for nn_KuramotoCoupling.

Reference computation (B=2, S=2048, D=1024, H=16, HD=64):
    mag   = sqrt(z_re^2 + z_im^2)
    q,k,v = mag @ W{q,k,v}.T  (per-head, HD=64)
    w     = softmax(q k^T / 8)
    value_mix    = (w @ v) @ Wo.T
    phase_update = c * (cos_ph * (w @ sin_ph) - sin_ph * (w @ cos_ph))
where sin_ph = z_im / mag, cos_ph = z_re / mag  (since ph = atan2(z_im, z_re)).

Sharding: 8 cores = 2 batches x 4 head-groups (4 heads each).  Each core
gets its batch's z (full D, column-permuted so its OWN 256 channels come
first), its heads' slices of Wq/Wk/Wv (rows permuted to match, bf16), and
its j-slice of Wo^T (bf16).  Outputs: phase slice [S, 256] (concatenated
on host) and a value_mix partial [S, D] (summed over head-groups on host).

Device-side structure per core:
  A) stream z by 128-row chunks: ssq = re^2+im^2 (gpsimd square + ACT
     square + DVE add); own channels = ssq[:, 0:256] -> rinv -> sin/cos
     into X tiles [s, (head: v|sin|cos|one)] (bf16); PE-transpose ssq and
     evict through ACT Sqrt -> magT [d, s] (bf16).
  B) qT/kT = WqT.T @ magT  [ch, s] (bf16), v = magT.T @ WvT -> X v cols.
  C) flat software pipeline over 8 units (2 q-tiles x 4 heads): for unit
     N+1 the scoresT = kT.T @ qT -> ACT exp (bf16 wT, no max-sub:
     |scores/8| < ~6) interleaved at k-granularity with unit N's mixing
     out[q, v|sin|cos|one] += wT.T @ X (accumulated over all 16 k in
     PSUM).  The one-column gives the softmax denominator.  Per-u DVE
     epilogue: recip, value rows scaled, phase = (Ksin*rc)*cos -
     (Kcos*rc)*sin.  PE and ACT overlap across units.
  D) per q-tile: PE-transpose value rows (bf16) -> vT, value_mix
     partial [q, i] = vT.T @ WoT, DMA'd straight from PSUM.
"""
import sys

if "/opt/trn_rl_repo" not in sys.path:
    sys.path.insert(0, "/opt/trn_rl_repo")

import numpy as np
import ml_dtypes
from contextlib import ExitStack

import concourse.bacc as bacc
import concourse.mybir as mybir
import concourse.tile as tile
from concourse.bass_utils import run_bass_kernel_spmd
from concourse.masks import make_identity

F32 = mybir.dt.float32
F32R = mybir.dt.float32r
BF16 = mybir.dt.bfloat16
AF = mybir.ActivationFunctionType
ALU = mybir.AluOpType

B, S, D, H = 2, 2048, 1024, 16
HD = D // H
N_CORES = 8
HG = 4                   # head-groups (tensor-parallel dim)
HPC = H // HG            # heads per core = 4
CH = HPC * HD            # own channels per core = 256
P = 128
SC = S // P              # 16 s-chunks
DC = D // P              # 8 d-chunks
XW = HPC * 256           # X tile width: per head [v64|sin64|cos64|one..]
XN = 193                 # mixing moving width: v64|sin64|cos64|one


def build_kernel(sc=SC):
    """Build the SPMD program for one core.  sc = number of 128-row
    s-chunks (16 for the real problem; smaller for debug builds)."""
    s = sc * P
    kc = sc                      # k-chunks == s-chunks
    uq = max(1, sc // 2)         # q-chunks per q-tile
    n_qt = 2 if sc >= 2 else 1
    qw = uq * P
    kpu = kc // uq               # score k-chunks issued per u-step (2)
    halves = ((0, sc // 2), (sc // 2, sc)) if sc >= 4 else ((0, sc),)

    nc = bacc.Bacc("TRN2", target_bir_lowering=False, debug=False,
                   num_devices=N_CORES)

    z_re = nc.dram_tensor("z_re", [s, D], F32, kind="ExternalInput").ap()
    z_im = nc.dram_tensor("z_im", [s, D], F32, kind="ExternalInput").ap()
    wq_t = nc.dram_tensor("wq_t", [D, CH], BF16, kind="ExternalInput").ap()
    wk_t = nc.dram_tensor("wk_t", [D, CH], BF16, kind="ExternalInput").ap()
    wv_t = nc.dram_tensor("wv_t", [D, CH], BF16, kind="ExternalInput").ap()
    wo_t = nc.dram_tensor("wo_t", [CH, D], BF16, kind="ExternalInput").ap()
    coup = nc.dram_tensor("coup", [P, 1], F32, kind="ExternalInput").ap()
    phase_out = nc.dram_tensor("phase_out", [s, CH], F32,
                               kind="ExternalOutput").ap()
    vmix_out = nc.dram_tensor("vmix_out", [s, D], F32,
                              kind="ExternalOutput").ap()

    with ExitStack() as ctx:
        tc = ctx.enter_context(tile.TileContext(nc))

        # ---- persistent pools -------------------------------------------
        const_p = ctx.enter_context(tc.tile_pool(name="const", bufs=1))
        wo_pool = ctx.enter_context(tc.tile_pool(name="wo", bufs=1))
        x_pool = ctx.enter_context(tc.tile_pool(name="x", bufs=1))
        qk_pool = ctx.enter_context(tc.tile_pool(name="qk", bufs=1))

        ident = const_p.tile([P, P], F32, tag="ident")
        make_identity(nc, ident[:])
        identb = const_p.tile([P, P], BF16, tag="identb")
        make_identity(nc, identb[:])
        coup_sb = const_p.tile([P, 1], F32, tag="coup")
        nc.sync.dma_start(coup_sb[:], coup[:, :])

        wo_sb = wo_pool.tile([P, 2 * D], BF16, tag="wo")
        nc.sync.dma_start(
            wo_sb[:].rearrange("p (j c) -> p j c", c=D),
            wo_t.rearrange("(j p) c -> p j c", p=P))

        # X tiles: per s-chunk [128, XW] bf16, per head [v|sin|cos|one]
        x_tiles = [x_pool.tile([P, XW], BF16, tag=f"x{i}", name=f"x{i}")
                   for i in range(sc)]
        # qT/kT: [ch, s] bf16, 2 chunk-tiles each (heads 0,1 / heads 2,3)
        qT = [qk_pool.tile([P, s], BF16, tag=f"qT{c}", name=f"qT{c}")
              for c in range(2)]
        kT = [qk_pool.tile([P, s], BF16, tag=f"kT{c}", name=f"kT{c}")
              for c in range(2)]

        # ---- phase A/B: mag, transposes, QKV ----------------------------
        with tc.tile_pool(name="wqkv", bufs=1) as w_pool, \
             tc.tile_pool(name="zb", bufs=2) as z_pool, \
             tc.tile_pool(name="sq", bufs=2) as sq_pool, \
             tc.tile_pool(name="ssq", bufs=3) as ssq_pool, \
             tc.tile_pool(name="rv", bufs=2) as rv_pool, \
             tc.tile_pool(name="magt", bufs=DC + 1) as mag_pool, \
             tc.tile_pool(name="tp", bufs=3, space="PSUM") as tp_pool, \
             tc.tile_pool(name="pqk", bufs=3, space="PSUM") as pqk_pool, \
             tc.tile_pool(name="pv", bufs=2, space="PSUM") as pv_pool:

            # weights: one DMA each, chunked [128, .] by contraction rows
            wq_sb = w_pool.tile([P, DC * CH], BF16, tag="wq")
            wk_sb = w_pool.tile([P, DC * CH], BF16, tag="wk")
            wv_sb = w_pool.tile([P, DC * CH], BF16, tag="wv")
            for wsb, wdr in ((wq_sb, wq_t), (wk_sb, wk_t), (wv_sb, wv_t)):
                nc.sync.dma_start(
                    wsb[:].rearrange("p (j c) -> p j c", c=CH),
                    wdr.rearrange("(j p) c -> p j c", p=P))

            for h0, h1 in halves:
                nh = h1 - h0
                ssq_tiles = {}
                # -- elementwise + sin/cos per s-chunk
                for si in range(h0, h1):
                    zr = z_pool.tile([P, D], F32, tag="zr")
                    zi = z_pool.tile([P, D], F32, tag="zi")
                    nc.sync.dma_start(zr[:], z_re[si * P:(si + 1) * P, :])
                    nc.sync.dma_start(zi[:], z_im[si * P:(si + 1) * P, :])
                    t1 = sq_pool.tile([P, D], F32, tag="t1")
                    t2 = sq_pool.tile([P, D], F32, tag="t2")
                    nc.gpsimd.tensor_mul(t1[:], zr[:], zr[:])
                    nc.scalar.activation(t2[:], zi[:], AF.Square)
                    ssq = ssq_pool.tile([P, D], F32, tag="ssq")
                    nc.vector.tensor_add(ssq[:], t1[:], t2[:])
                    ssq_tiles[si] = ssq

                    # own channels are the first CH columns (host-permuted)
                    msl = rv_pool.tile([P, CH], F32, tag="msl")
                    rinv = rv_pool.tile([P, CH], F32, tag="rinv")
                    nc.scalar.activation(msl[:], ssq[:, 0:CH], AF.Sqrt)
                    nc.vector.reciprocal_approx_fast(out=rinv[:], in_=msl[:])

                    x3 = x_tiles[si][:].rearrange("p (h c) -> p h c", c=256)
                    z3r = zr[:, 0:CH].rearrange("p (h c) -> p h c", c=HD)
                    z3i = zi[:, 0:CH].rearrange("p (h c) -> p h c", c=HD)
                    r3 = rinv[:].rearrange("p (h c) -> p h c", c=HD)
                    nc.vector.tensor_mul(x3[:, :, 64:128], z3i[:], r3[:])
                    nc.vector.tensor_mul(x3[:, :, 128:192], z3r[:], r3[:])
                    nc.vector.tensor_scalar(
                        out=x3[:, :, 192:193], in0=r3[:, :, 0:1],
                        scalar1=0.0, scalar2=1.0, op0=ALU.mult, op1=ALU.add)

                # -- transposes of ssq -> magT (ACT sqrt evict), groups of 2
                mag_t = [mag_pool.tile([P, nh * P], BF16, tag="magt",
                                       name=f"magt{j}")
                         for j in range(DC)]
                for g in range(0, nh, 2):
                    sa, sb_ = h0 + g, h0 + g + 1
                    for j in range(DC):
                        tp = tp_pool.tile([P, 256], F32, tag="tp")
                        nc.tensor.transpose(
                            tp[:, 0:128], ssq_tiles[sa][:, j * P:(j + 1) * P],
                            ident[:])
                        if sb_ < h1:
                            nc.tensor.transpose(
                                tp[:, 128:256],
                                ssq_tiles[sb_][:, j * P:(j + 1) * P], ident[:])
                        nc.scalar.activation(
                            mag_t[j][:, g * P:(g + 2) * P], tp[:], AF.Sqrt)

                # -- qT / kT for this half
                for c in range(2):
                    for t0 in range(0, nh * P, 512):
                        tw = min(512, nh * P - t0)
                        for wsb, dst in ((wq_sb, qT[c]), (wk_sb, kT[c])):
                            ps = pqk_pool.tile([P, 512], F32, tag="pqk")
                            for j in range(DC):
                                nc.tensor.matmul(
                                    ps[:, 0:tw],
                                    wsb[:, j * CH + c * P:j * CH + (c + 1) * P],
                                    mag_t[j][:, t0:t0 + tw],
                                    start=(j == 0), stop=(j == DC - 1))
                            nc.vector.tensor_copy(
                                dst[:, h0 * P + t0:h0 * P + t0 + tw],
                                ps[:, 0:tw])

                # -- v for this half -> X v cols
                for ii in range(nh):
                    si = h0 + ii
                    psv = pv_pool.tile([P, CH], F32, tag="pv")
                    for j in range(DC):
                        nc.tensor.matmul(
                            psv[:], mag_t[j][:, ii * P:(ii + 1) * P],
                            wv_sb[:, j * CH:(j + 1) * CH],
                            start=(j == 0), stop=(j == DC - 1))
                    x3 = x_tiles[si][:].rearrange("p (h c) -> p h c", c=256)
                    p3 = psv[:].rearrange("p (h c) -> p h c", c=HD)
                    nc.vector.tensor_copy(x3[:, :, 0:HD], p3[:])

        # ---- phase C/D: pipelined attention + output --------------------
        units = [(qt, h) for qt in range(n_qt) for h in range(HPC)]
        n_units = len(units)

        with tc.tile_pool(name="wt", bufs=2 * kc) as wt_pool, \
             tc.tile_pool(name="vout", bufs=2 * uq) as vout_pool, \
             tc.tile_pool(name="vT", bufs=2) as vT_pool, \
             tc.tile_pool(name="ph", bufs=2 * uq) as ph_pool, \
             tc.tile_pool(name="vm", bufs=2) as vm_pool, \
             tc.tile_pool(name="rc", bufs=8) as rc_pool, \
             tc.tile_pool(name="tmp", bufs=4) as tmp_pool, \
             tc.tile_pool(name="pss", bufs=3, space="PSUM") as pss_pool, \
             tc.tile_pool(name="pso", bufs=2, space="PSUM") as pso_pool, \
             tc.tile_pool(name="pd", bufs=2, space="PSUM") as pd_pool:

            wt_sets = [None] * n_units
            vout_sets = {}
            ph_sets = {}

            def issue_scores(su, k, wt):
                qt, h = units[su]
                c, hh = divmod(h, 2)
                q0 = (qt * uq) * P
                for qh in range(0, qw, 512):
                    qn = min(512, qw - qh)
                    ps = pss_pool.tile([P, 512], F32, tag="pss", name="pss")
                    nc.tensor.matmul(
                        ps[:, 0:qn],
                        kT[c][hh * 64:(hh + 1) * 64, k * P:(k + 1) * P],
                        qT[c][hh * 64:(hh + 1) * 64, q0 + qh:q0 + qh + qn],
                        start=True, stop=True)
                    nc.scalar.activation(wt[:, qh:qh + qn], ps[:, 0:qn],
                                         AF.Exp, scale=0.125)

            def issue_mix(pu, u, ps, off):
                qt, h = units[pu]
                wts = wt_sets[pu]
                for k in range(kc):
                    nc.tensor.matmul(
                        ps[:, off:off + XN],
                        wts[k][:, u * P:(u + 1) * P],
                        x_tiles[k][:, h * 256:h * 256 + XN],
                        start=(k == 0), stop=(k == kc - 1))

            def issue_epilogue(pu, u, ps, off):
                qt, h = units[pu]
                si = qt * uq + u
                rec = rc_pool.tile([P, 1], F32, tag="rec", name="rec")
                rcp = rc_pool.tile([P, 1], F32, tag="rcp", name="rcp")
                nc.vector.reciprocal(rec[:], ps[:, off + 192:off + 193])
                nc.vector.tensor_scalar_mul(rcp[:], rec[:], coup_sb[:])
                nc.vector.tensor_scalar(
                    out=vout_sets[qt][u][:, h * HD:(h + 1) * HD],
                    in0=ps[:, off:off + HD], scalar1=rec[:], scalar2=None,
                    op0=ALU.mult)
                x3 = x_tiles[si][:].rearrange("p (h c) -> p h c", c=256)
                cos_src = x3[:, h, 128:192]
                sin_src = x3[:, h, 64:128]
                ta = tmp_pool.tile([P, HD], F32, tag="ta", name="ta")
                tb = tmp_pool.tile([P, HD], F32, tag="tb", name="tb")
                nc.vector.scalar_tensor_tensor(
                    out=ta[:], in0=ps[:, off + 64:off + 128], scalar=rcp[:],
                    in1=cos_src, op0=ALU.mult, op1=ALU.mult)
                nc.vector.scalar_tensor_tensor(
                    out=tb[:], in0=ps[:, off + 128:off + 192], scalar=rcp[:],
                    in1=sin_src, op0=ALU.mult, op1=ALU.mult)
                nc.vector.tensor_sub(
                    ph_sets[qt][u][:, h * HD:(h + 1) * HD], ta[:], tb[:])
                if h == HPC - 1:
                    nc.sync.dma_start(
                        phase_out[si * P:(si + 1) * P, :], ph_sets[qt][u][:])

            def issue_phase_d(qt):
                t0 = qt * uq
                vT = [vT_pool.tile([P, qw], BF16, tag=f"vT{c}",
                                   name=f"vT{c}") for c in range(2)]
                for c in range(2):
                    for g in range(0, uq, 4):
                        gn = min(4, uq - g)
                        tp = pd_pool.tile([P, 512], BF16, tag="pdT",
                                          name="pdT", bufs=1)
                        for u in range(g, g + gn):
                            nc.tensor.transpose(
                                tp[:, (u - g) * P:(u - g + 1) * P],
                                vout_sets[qt][u][:, c * P:(c + 1) * P],
                                identb[:])
                        nc.vector.tensor_copy(
                            vT[c][:, g * P:(g + gn) * P], tp[:, 0:gn * P])
                for u in range(uq):
                    su_ = t0 + u
                    vm = vm_pool.tile([P, D], F32, tag="vm", name="vm")
                    for iti, it in enumerate(range(0, D, 512)):
                        psm = pd_pool.tile([P, 512], F32, tag="pdM",
                                           name="pdM")
                        for c in range(2):
                            nc.tensor.matmul(
                                psm[:],
                                vT[c][:, u * P:(u + 1) * P],
                                wo_sb[:, c * D + it:c * D + it + 512],
                                start=(c == 0), stop=(c == 1))
                        # split evictions DVE/ACT to balance engine load
                        if iti % 2 == 0:
                            nc.vector.tensor_copy(
                                vm[:, it:it + 512], psm[:])
                        else:
                            nc.scalar.activation(
                                vm[:, it:it + 512], psm[:], AF.Copy)
                    nc.sync.dma_start(
                        vmix_out[su_ * P:(su_ + 1) * P, :], vm[:])

            for s2 in range(n_units + 1):
                cur = s2 if s2 < n_units else None
                prev = s2 - 1 if s2 > 0 else None
                if cur is not None:
                    wt_sets[cur] = [
                        wt_pool.tile([P, qw], BF16, tag="wt",
                                     name=f"wt{cur}_{k}")
                        for k in range(kc)]
                if prev is not None and units[prev][1] == 0:
                    pqt = units[prev][0]
                    vout_sets[pqt] = [
                        vout_pool.tile([P, CH], BF16, tag="vout",
                                       name=f"vo{pqt}_{u}")
                        for u in range(uq)]
                    ph_sets[pqt] = [
                        ph_pool.tile([P, CH], F32, tag="ph",
                                     name=f"ph{pqt}_{u}")
                        for u in range(uq)]
                psos = {}
                for u in range(uq):
                    if cur is not None:
                        for k in range(u * kpu, (u + 1) * kpu):
                            issue_scores(cur, k, wt_sets[cur][k])
                    if prev is not None:
                        j, off = u // 2, (u % 2) * 256
                        if u % 2 == 0:
                            psos[j] = pso_pool.tile([P, 512], F32,
                                                    tag="pso", name="pso")
                        issue_mix(prev, u, psos[j], off)
                        issue_epilogue(prev, u, psos[j], off)
                if prev is not None and units[prev][1] == HPC - 1:
                    issue_phase_d(units[prev][0])

    if not nc.is_finalized():
        nc.finalize()
    return nc


_CACHED = {}


def _get_nc(sc=SC):
    if sc not in _CACHED:
        _CACHED[sc] = build_kernel(sc)
    return _CACHED[sc]


def make_in_maps(z_re, z_im, Wq, Wk, Wv, Wo, coupling_strength, sc=SC):
    """Host-side sharding: core = b * HG + hg.  z columns (and W{q,k,v}
    contraction rows) are permuted so the core's own channels come first."""
    z_re = np.asarray(z_re, dtype=np.float32)
    z_im = np.asarray(z_im, dtype=np.float32)
    Wq = np.asarray(Wq, dtype=np.float32)
    Wk = np.asarray(Wk, dtype=np.float32)
    Wv = np.asarray(Wv, dtype=np.float32)
    Wo = np.asarray(Wo, dtype=np.float32)
    coup_b = np.full((P, 1), np.float32(np.asarray(coupling_strength)),
                     dtype=np.float32)
    s = sc * P
    bf = ml_dtypes.bfloat16
    in_maps = []
    for core in range(N_CORES):
        b, hg = divmod(core, HG)
        lo, hi = hg * CH, (hg + 1) * CH
        perm = np.r_[lo:hi, 0:lo, hi:D]
        in_maps.append({
            "z_re": np.ascontiguousarray(z_re[b, :s][:, perm]),
            "z_im": np.ascontiguousarray(z_im[b, :s][:, perm]),
            "wq_t": np.ascontiguousarray(Wq[lo:hi][:, perm].T.astype(bf)),
            "wk_t": np.ascontiguousarray(Wk[lo:hi][:, perm].T.astype(bf)),
            "wv_t": np.ascontiguousarray(Wv[lo:hi][:, perm].T.astype(bf)),
            "wo_t": np.ascontiguousarray(Wo[:, lo:hi].T.astype(bf)),
            "coup": coup_b,
        })
    return in_maps


def gather_outputs(results, sc=SC):
    s = sc * P
    phase = np.zeros((B, s, D), dtype=np.float32)
    vmix = np.zeros((B, s, D), dtype=np.float32)
    for core in range(N_CORES):
        b, hg = divmod(core, HG)
        lo, hi = hg * CH, (hg + 1) * CH
        phase[b, :, lo:hi] = results[core]["phase_out"]
        vmix[b] += results[core]["vmix_out"]
    return phase, vmix


def kernel(z_re, z_im, Wq, Wk, Wv, Wo, coupling_strength):
    nc = _get_nc(SC)
    in_maps = make_in_maps(z_re, z_im, Wq, Wk, Wv, Wo, coupling_strength)
    res = run_bass_kernel_spmd(nc, in_maps, core_ids=list(range(N_CORES)))
    return gather_outputs(res.results)
